# revision 8
# baseline (speedup 1.0000x reference)
"""Multi-head attention (B=2, S=4096, D=1024, H=16) on 8 NeuronCores.

Sharding: core c = (batch b = c // 4, head-group g = c % 4).  Each head-group
owns 4 heads = 256 projection features.

v3: fp16 operands everywhere (single-pass PE matmuls, 10-bit mantissa keeps
rel err ~1e-3); all transposes/casts done on the HOST (inputs ship as
qT/kT/vT [D, S] fp16, wqT/wkT/wvT [D, E], w0T [E, D]); batched 3D-AP DMA
loads; softmax normalization fused into the PSUM->SBUF eviction of the PV
accumulator (reciprocal of the ones-column row, partition-broadcast,
tensor_tensor multiply) so there is no transpose-based normalize phase; the
output projection for a q-block is emitted right after its 4 heads finish,
overlapping the next block's attention.  Host sums the 4 fp16 partials per
batch in fp32.
"""

import numpy as np
from contextlib import ExitStack

import concourse.bass as bass
import concourse.bacc as bacc
import concourse.tile as tile
from concourse import mybir, bass_utils

B, S, D, H = 2, 4096, 1024, 16
DK = D // H          # 64
NCORES = 8
GROUPS = 4           # head-groups (tensor parallel)
HG = H // GROUPS     # 4 heads per group
E = HG * DK          # 256 features per group

F32 = mybir.dt.float32
F16 = mybir.dt.float16

P = 128              # partitions
SC = S // P          # 32 s-chunks of 128
SG = 8               # s-groups in projection
SGW = S // SG        # 512
DC = D // P          # 8 d-chunks
QB = 1024            # q-block in attention
NQB = S // QB        # 4
QC = QB // P         # 8 q-chunks per block
NST = SC             # 32 k-stripes of 128
VW = DK + 1          # vp columns per head incl. ones column (65)
VPAD = 66            # padded per-head stride in vps tile


def kernel_body(tc, qT, kT, vT, wqT, wkT, wvT, w0T, out):
    nc = tc.nc
    ctx = ExitStack()
    with ctx:
        # persistent weights (pre-transposed on host, loaded in 4 DMAs)
        w_pool = ctx.enter_context(tc.tile_pool(name="wsb", bufs=1))
        w0sb = w_pool.tile([P, 2, D], F16, tag="w0sb", name="w0sb")
        wqsb = w_pool.tile([P, DC, E], F16, tag="wqsb", name="wqsb")
        wksb = w_pool.tile([P, DC, E], F16, tag="wksb", name="wksb")
        wvsb = w_pool.tile([P, DC, E], F16, tag="wvsb", name="wvsb")
        nc.sync.dma_start(out=w0sb,
                          in_=w0T.rearrange("(c p) d -> p c d", p=P))
        for wsrc, wdst in ((wqT, wqsb), (wkT, wksb), (wvT, wvsb)):
            nc.sync.dma_start(out=wdst,
                              in_=wsrc.rearrange("(c p) e -> p c e", p=P))

        # persistent through phase A
        proj_ctx = ExitStack()
        proj_pool = proj_ctx.enter_context(tc.tile_pool(name="proj", bufs=1))
        qpT = [proj_pool.tile([P, S], F16, tag=f"qpT{i}", name=f"qpT{i}")
               for i in range(2)]
        kpT = [proj_pool.tile([P, S], F16, tag=f"kpT{i}", name=f"kpT{i}")
               for i in range(2)]
        vps = proj_pool.tile([P, SC, HG * VPAD], F16, tag="vps", name="vps")

        qTr = qT.rearrange("(c p) s -> p c s", p=P)
        kTr = kT.rearrange("(c p) s -> p c s", p=P)
        vTr = vT.rearrange("(c p) s -> p c s", p=P)

        # ================= phase P: projections =================
        with tc.tile_pool(name="p_xin", bufs=3) as xin_pool, \
             tc.tile_pool(name="p_psum", bufs=3, space="PSUM") as ppool_a:
            for sg in range(SG):
                s0 = sg * SGW
                qg = xin_pool.tile([P, DC, SGW], F16, tag="qg", name="qg")
                kg = xin_pool.tile([P, DC, SGW], F16, tag="kg", name="kg")
                vg = xin_pool.tile([P, DC, SGW], F16, tag="vg", name="vg")
                nc.sync.dma_start(out=qg, in_=qTr[:, :, s0:s0 + SGW])
                nc.sync.dma_start(out=kg, in_=kTr[:, :, s0:s0 + SGW])
                nc.sync.dma_start(out=vg, in_=vTr[:, :, s0:s0 + SGW])
                for xg, wsb, dst in ((qg, wqsb, qpT), (kg, wksb, kpT)):
                    for et in range(2):
                        acc = ppool_a.tile([P, SGW], F32, tag="acc",
                                           name="acc")
                        for dc in range(DC):
                            nc.tensor.matmul(
                                acc,
                                wsb[:, dc, et * P:(et + 1) * P],
                                xg[:, dc, :],
                                start=(dc == 0), stop=(dc == DC - 1))
                        nc.vector.tensor_copy(
                            out=dst[et][:, s0:s0 + SGW], in_=acc)
                for sc4 in range(SGW // P):
                    scg = sg * (SGW // P) + sc4
                    accv = ppool_a.tile([P, E], F32, tag="accv", name="accv")
                    for dc in range(DC):
                        nc.tensor.matmul(
                            accv,
                            vg[:, dc, sc4 * P:(sc4 + 1) * P],
                            wvsb[:, dc, :],
                            start=(dc == 0), stop=(dc == DC - 1))
                    for h in range(HG):
                        nc.vector.tensor_copy(
                            out=vps[:, scg, h * VPAD:h * VPAD + DK],
                            in_=accv[:, h * DK:(h + 1) * DK])
            # ones column for the PV sums row
            ones_sc = xin_pool.tile([P, SC], F16, tag="ones_sc",
                                    name="ones_sc")
            nc.vector.memset(ones_sc, 1.0)
            for h in range(HG):
                nc.vector.tensor_copy(
                    out=vps[:, :, h * VPAD + DK:h * VPAD + DK + 1],
                    in_=ones_sc.rearrange("p (s o) -> p s o", o=1))

        # ============ phase A: attention + interleaved out-proj ============
        xw_pool = ctx.enter_context(
            tc.tile_pool(name="xw", bufs=1, side="right"))
        xw = [xw_pool.tile([P, S], F16, tag=f"xw{i}", name=f"xw{i}")
              for i in range(2)]
        with tc.tile_pool(name="a_att", bufs=3) as att_pool, \
             tc.tile_pool(name="a_xr", bufs=2) as xr_pool, \
             tc.tile_pool(name="a_rn", bufs=2) as rn_pool, \
             tc.tile_pool(name="a_osb", bufs=2) as osb_pool, \
             tc.tile_pool(name="a_st", bufs=2, space="PSUM") as ppool_st, \
             tc.tile_pool(name="a_x", bufs=1, space="PSUM") as ppool_x, \
             tc.tile_pool(name="a_w", bufs=2, space="PSUM") as ppool_w:

            def emit_wproj(qb, qc):
                # out-projection for q-chunk qc of block qb (xw already final)
                qq = qb * QB + qc * P
                osb = osb_pool.tile([P, D], F16, tag="osb", name="osb")
                for j in range(2):
                    oacc = ppool_w.tile([P, 512], F32, tag="oacc",
                                        name="oacc")
                    for ec in range(2):
                        nc.tensor.matmul(
                            oacc,
                            xw[ec][:, qq:qq + P],
                            w0sb[:, ec, j * 512:(j + 1) * 512],
                            start=(ec == 0), stop=(ec == 1))
                    nc.vector.tensor_copy(
                        out=osb[:, j * 512:(j + 1) * 512], in_=oacc)
                nc.sync.dma_start(out=out[qq:qq + P, :], in_=osb)

            for qb in range(NQB):
                q0 = qb * QB
                for h in range(HG):
                    et, hp = h // 2, (h % 2) * DK
                    xacc = ppool_x.tile([VW, QB], F32, tag="xacc",
                                        name="xacc")

                    def emit_scores(kk):
                        lhs_k = kpT[et][hp:hp + DK, kk * P:(kk + 1) * P]
                        st = ppool_st.tile([P, QB], F32, tag="st", name="st")
                        for j in range(2):
                            nc.tensor.matmul(
                                st[:, j * 512:(j + 1) * 512],
                                lhs_k,
                                qpT[et][hp:hp + DK,
                                        q0 + j * 512:q0 + (j + 1) * 512],
                                start=True, stop=True)
                        return st

                    # software-pipelined: scores for kk+1 are queued on the
                    # PE before PV of kk (which waits on exp) so the in-order
                    # PE queue never head-of-line blocks on the ACT engine
                    st = emit_scores(0)
                    for kk in range(NST):
                        attst = att_pool.tile([P, QB], F16, tag="att",
                                              name="att")
                        nc.scalar.activation(
                            attst, st, mybir.ActivationFunctionType.Exp,
                            scale=0.125)
                        if kk + 1 < NST:
                            st = emit_scores(kk + 1)
                        lhs_v = vps[:, kk, h * VPAD:h * VPAD + VW]
                        for j in range(2):
                            nc.tensor.matmul(
                                xacc[:, j * 512:(j + 1) * 512],
                                lhs_v,
                                attst[:, j * 512:(j + 1) * 512],
                                start=(kk == 0), stop=(kk == NST - 1))
                        # spread the previous block's out-projection between
                        # k-stripes of the first head so the PE never queues
                        # a long W burst that would starve the ACT pipeline
                        if h == 0 and qb > 0 and kk % 4 == 3:
                            emit_wproj(qb - 1, kk // 4)
                    # evict the PV accumulator quickly (frees PSUM for the
                    # next head), then normalize by 1/rowsum off to the side
                    xraw = xr_pool.tile([VW, QB], F16, tag="xraw",
                                        name="xraw")
                    nc.vector.tensor_copy(out=xraw, in_=xacc)
                    rcp = rn_pool.tile([1, QB], F32, tag="rcp", name="rcp")
                    nc.vector.reciprocal(rcp, xraw[DK:DK + 1, :])
                    rcp16 = rn_pool.tile([1, QB], F16, tag="rcp16",
                                         name="rcp16")
                    nc.vector.tensor_copy(out=rcp16, in_=rcp)
                    rbc = rn_pool.tile([DK, QB], F16, tag="rbc", name="rbc")
                    nc.gpsimd.partition_broadcast(rbc, rcp16)
                    nc.vector.tensor_tensor(
                        xw[et][hp:hp + DK, q0:q0 + QB],
                        xraw[0:DK, :], rbc, mybir.AluOpType.mult)
            for qc in range(QC):
                emit_wproj(NQB - 1, qc)
        proj_ctx.close()


def build_program():
    nc = bacc.Bacc("TRN2", target_bir_lowering=False, debug=False,
                   num_devices=NCORES)
    qT = nc.dram_tensor("qT", (D, S), F16, kind="ExternalInput").ap()
    kT = nc.dram_tensor("kT", (D, S), F16, kind="ExternalInput").ap()
    vT = nc.dram_tensor("vT", (D, S), F16, kind="ExternalInput").ap()
    wqT = nc.dram_tensor("wqT", (D, E), F16, kind="ExternalInput").ap()
    wkT = nc.dram_tensor("wkT", (D, E), F16, kind="ExternalInput").ap()
    wvT = nc.dram_tensor("wvT", (D, E), F16, kind="ExternalInput").ap()
    w0T = nc.dram_tensor("w0T", (E, D), F16, kind="ExternalInput").ap()
    out = nc.dram_tensor("out", (S, D), F16, kind="ExternalOutput").ap()
    with tile.TileContext(nc) as tc:
        kernel_body(tc, qT, kT, vT, wqT, wkT, wvT, w0T, out)
    nc.compile()
    return nc


_NC_CACHE = None


def _get_program():
    global _NC_CACHE
    if _NC_CACHE is None:
        _NC_CACHE = build_program()
    return _NC_CACHE


def make_in_maps(q, k, v, wq, wk, wv, w0):
    arrs = [np.asarray(a, dtype=np.float32)
            for a in (q, k, v, wq, wk, wv, w0)]
    q, k, v, wq, wk, wv, w0 = arrs
    f16 = np.float16
    # per-batch transposed activations (shared by the 4 cores of a batch)
    qTb = [np.ascontiguousarray(q[b].T).astype(f16) for b in range(B)]
    kTb = [np.ascontiguousarray(k[b].T).astype(f16) for b in range(B)]
    vTb = [np.ascontiguousarray(v[b].T).astype(f16) for b in range(B)]
    in_maps = []
    for c in range(NCORES):
        b, g = c // GROUPS, c % GROUPS
        e0 = g * E
        in_maps.append({
            "qT": qTb[b],
            "kT": kTb[b],
            "vT": vTb[b],
            "wqT": np.ascontiguousarray(wq[e0:e0 + E, :].T).astype(f16),
            "wkT": np.ascontiguousarray(wk[e0:e0 + E, :].T).astype(f16),
            "wvT": np.ascontiguousarray(wv[e0:e0 + E, :].T).astype(f16),
            "w0T": np.ascontiguousarray(w0[:, e0:e0 + E].T).astype(f16),
        })
    return in_maps


def gather_out(results):
    out = np.zeros((B, S, D), dtype=np.float32)
    for c in range(NCORES):
        b = c // GROUPS
        out[b] += results[c]["out"].astype(np.float32)
    return out


def _install_ntff_hook_shim():
    """This image's antenv lacks axon_hooks; recreate it so trace=True works.

    Mirrors trn_agent_boot.trn_boot._ntff_profile_via_ctypes against
    /opt/axon/libaxon_pjrt.so.
    """
    import sys, types, ctypes, contextlib
    if "antenv.axon_hooks" in sys.modules:
        return
    mod = types.ModuleType("antenv.axon_hooks")
    mod._hook = None

    def set_axon_ntff_profile_hook(h):
        mod._hook = h

    def get_axon_ntff_profile_hook():
        return mod._hook

    mod.set_axon_ntff_profile_hook = set_axon_ntff_profile_hook
    mod.get_axon_ntff_profile_hook = get_axon_ntff_profile_hook
    sys.modules["antenv.axon_hooks"] = mod
    try:
        import antenv
        antenv.axon_hooks = mod
    except ImportError:
        pass

    so_path = "/opt/axon/libaxon_pjrt.so"
    try:
        lib = ctypes.CDLL(so_path)
        if not hasattr(lib, "axon_start_nrt_profile"):
            return
        lib.axon_start_nrt_profile.argtypes = [
            ctypes.POINTER(ctypes.c_int64), ctypes.c_size_t]
        lib.axon_start_nrt_profile.restype = ctypes.c_int64
        lib.axon_stop_nrt_profile.argtypes = [ctypes.c_char_p]
        lib.axon_stop_nrt_profile.restype = ctypes.c_int64
    except OSError:
        return

    @contextlib.contextmanager
    def _hook(output_dir, device_ids):
        import jax
        jax.devices()
        if device_ids:
            ids = (ctypes.c_int64 * len(device_ids))(*device_ids)
            rc = lib.axon_start_nrt_profile(ids, len(device_ids))
        else:
            rc = lib.axon_start_nrt_profile(None, 0)
        if rc != 0:
            raise RuntimeError(f"axon_start_nrt_profile rc={rc}")
        try:
            yield
        finally:
            n = lib.axon_stop_nrt_profile(str(output_dir).encode())
            print(f"profile: {n} file(s) written to {output_dir}")

    mod._hook = _hook


def kernel(q, k, v, wq, wk, wv, w0, _trace=False, _tmpdir=None):
    if _trace:
        _install_ntff_hook_shim()
    nc = _get_program()
    in_maps = make_in_maps(q, k, v, wq, wk, wv, w0)
    res = bass_utils.run_bass_kernel_spmd(
        nc, in_maps, core_ids=list(range(NCORES)),
        trace=_trace, tmpdir=_tmpdir)
    out = gather_out(res.results)
    if _trace:
        return out, res
    return out


# revision 10
# speedup vs baseline: 1.0777x; 1.0777x over previous
"""Multi-head attention (B=2, S=4096, D=1024, H=16) on 8 NeuronCores.

Sharding: core c = (batch b = c // 4, head-group g = c % 4).  Each head-group
owns 4 heads = 256 projection features.

v3: fp16 operands everywhere (single-pass PE matmuls, 10-bit mantissa keeps
rel err ~1e-3); all transposes/casts done on the HOST (inputs ship as
qT/kT/vT [D, S] fp16, wqT/wkT/wvT [D, E], w0T [E, D]); batched 3D-AP DMA
loads; softmax normalization fused into the PSUM->SBUF eviction of the PV
accumulator (reciprocal of the ones-column row, partition-broadcast,
tensor_tensor multiply) so there is no transpose-based normalize phase; the
output projection for a q-block is emitted right after its 4 heads finish,
overlapping the next block's attention.  Host sums the 4 fp16 partials per
batch in fp32.
"""

import numpy as np
from contextlib import ExitStack

import concourse.bass as bass
import concourse.bacc as bacc
import concourse.tile as tile
from concourse import mybir, bass_utils

B, S, D, H = 2, 4096, 1024, 16
DK = D // H          # 64
NCORES = 8
GROUPS = 4           # head-groups (tensor parallel)
HG = H // GROUPS     # 4 heads per group
E = HG * DK          # 256 features per group

F32 = mybir.dt.float32
F16 = mybir.dt.float16

P = 128              # partitions
SC = S // P          # 32 s-chunks of 128
SG = 8               # s-groups in projection
SGW = S // SG        # 512
DC = D // P          # 8 d-chunks
QB = 1024            # q-block in attention
NQB = S // QB        # 4
QC = QB // P         # 8 q-chunks per block
NST = SC             # 32 k-stripes of 128
VW = DK + 1          # vp columns per head incl. ones column (65)
VPAD = 66            # padded per-head stride in vps tile


def kernel_body(tc, qT, kT, vT, wqT, wkT, wvT, w0T, out):
    nc = tc.nc
    ctx = ExitStack()
    with ctx:
        # persistent weights (pre-transposed on host, loaded in 4 DMAs)
        w_pool = ctx.enter_context(tc.tile_pool(name="wsb", bufs=1))
        w0sb = w_pool.tile([P, 2, D], F16, tag="w0sb", name="w0sb")
        wqsb = w_pool.tile([P, DC, E], F16, tag="wqsb", name="wqsb")
        wksb = w_pool.tile([P, DC, E], F16, tag="wksb", name="wksb")
        wvsb = w_pool.tile([P, DC, E], F16, tag="wvsb", name="wvsb")
        nc.sync.dma_start(out=w0sb,
                          in_=w0T.rearrange("(c p) d -> p c d", p=P))
        for wsrc, wdst in ((wqT, wqsb), (wkT, wksb), (wvT, wvsb)):
            nc.sync.dma_start(out=wdst,
                              in_=wsrc.rearrange("(c p) e -> p c e", p=P))

        # persistent through phase A
        proj_ctx = ExitStack()
        proj_pool = proj_ctx.enter_context(tc.tile_pool(name="proj", bufs=1))
        qpT = [proj_pool.tile([P, S], F16, tag=f"qpT{i}", name=f"qpT{i}")
               for i in range(2)]
        kpT = [proj_pool.tile([P, S], F16, tag=f"kpT{i}", name=f"kpT{i}")
               for i in range(2)]
        vps = proj_pool.tile([P, SC, HG * VPAD], F16, tag="vps", name="vps")

        qTr = qT.rearrange("(c p) s -> p c s", p=P)
        kTr = kT.rearrange("(c p) s -> p c s", p=P)
        vTr = vT.rearrange("(c p) s -> p c s", p=P)

        # ================= phase P: projections =================
        with tc.tile_pool(name="p_xin", bufs=3) as xin_pool, \
             tc.tile_pool(name="p_psum", bufs=3, space="PSUM") as ppool_a:
            # k/v first: attention only needs kpT/vps in full plus the first
            # q-block of qpT, so q-projection for later blocks hides under
            # the ACT-bound start of attention
            for sg in range(SG):
                s0 = sg * SGW
                kg = xin_pool.tile([P, DC, SGW], F16, tag="kg", name="kg")
                vg = xin_pool.tile([P, DC, SGW], F16, tag="vg", name="vg")
                nc.sync.dma_start(out=kg, in_=kTr[:, :, s0:s0 + SGW])
                nc.sync.dma_start(out=vg, in_=vTr[:, :, s0:s0 + SGW])
                for et in range(2):
                    acc = ppool_a.tile([P, SGW], F32, tag="acc", name="acc")
                    for dc in range(DC):
                        nc.tensor.matmul(
                            acc,
                            wksb[:, dc, et * P:(et + 1) * P],
                            kg[:, dc, :],
                            start=(dc == 0), stop=(dc == DC - 1))
                    nc.vector.tensor_copy(
                        out=kpT[et][:, s0:s0 + SGW], in_=acc)
                for sc4 in range(SGW // P):
                    scg = sg * (SGW // P) + sc4
                    accv = ppool_a.tile([P, E], F32, tag="accv", name="accv")
                    for dc in range(DC):
                        nc.tensor.matmul(
                            accv,
                            vg[:, dc, sc4 * P:(sc4 + 1) * P],
                            wvsb[:, dc, :],
                            start=(dc == 0), stop=(dc == DC - 1))
                    for h in range(HG):
                        nc.vector.tensor_copy(
                            out=vps[:, scg, h * VPAD:h * VPAD + DK],
                            in_=accv[:, h * DK:(h + 1) * DK])
            # ones column for the PV sums row
            ones_sc = xin_pool.tile([P, SC], F16, tag="ones_sc",
                                    name="ones_sc")
            nc.vector.memset(ones_sc, 1.0)
            for h in range(HG):
                nc.vector.tensor_copy(
                    out=vps[:, :, h * VPAD + DK:h * VPAD + DK + 1],
                    in_=ones_sc.rearrange("p (s o) -> p s o", o=1))
            for sg in range(SG):
                s0 = sg * SGW
                qg = xin_pool.tile([P, DC, SGW], F16, tag="qg", name="qg")
                nc.sync.dma_start(out=qg, in_=qTr[:, :, s0:s0 + SGW])
                for et in range(2):
                    acc = ppool_a.tile([P, SGW], F32, tag="acc", name="acc")
                    for dc in range(DC):
                        nc.tensor.matmul(
                            acc,
                            wqsb[:, dc, et * P:(et + 1) * P],
                            qg[:, dc, :],
                            start=(dc == 0), stop=(dc == DC - 1))
                    nc.vector.tensor_copy(
                        out=qpT[et][:, s0:s0 + SGW], in_=acc)

        # ============ phase A: attention + interleaved out-proj ============
        xw_pool = ctx.enter_context(
            tc.tile_pool(name="xw", bufs=1, side="right"))
        xw = [xw_pool.tile([P, S], F16, tag=f"xw{i}", name=f"xw{i}")
              for i in range(2)]
        with tc.tile_pool(name="a_att", bufs=3) as att_pool, \
             tc.tile_pool(name="a_rn", bufs=2) as rn_pool, \
             tc.tile_pool(name="a_osb", bufs=2) as osb_pool, \
             tc.tile_pool(name="a_st", bufs=2, space="PSUM") as ppool_st, \
             tc.tile_pool(name="a_x", bufs=1, space="PSUM") as ppool_x, \
             tc.tile_pool(name="a_w", bufs=2, space="PSUM") as ppool_w:
            for qb in range(NQB):
                q0 = qb * QB
                for h in range(HG):
                    et, hp = h // 2, (h % 2) * DK
                    xacc = ppool_x.tile([VW, QB], F32, tag="xacc",
                                        name="xacc")
                    for kk in range(NST):
                        attst = att_pool.tile([P, QB], F16, tag="att",
                                              name="att")
                        lhs_k = kpT[et][hp:hp + DK, kk * P:(kk + 1) * P]
                        st = ppool_st.tile([P, QB], F32, tag="st", name="st")
                        for j in range(2):
                            nc.tensor.matmul(
                                st[:, j * 512:(j + 1) * 512],
                                lhs_k,
                                qpT[et][hp:hp + DK,
                                        q0 + j * 512:q0 + (j + 1) * 512],
                                start=True, stop=True)
                        nc.scalar.activation(
                            attst, st, mybir.ActivationFunctionType.Exp,
                            scale=0.125)
                        lhs_v = vps[:, kk, h * VPAD:h * VPAD + VW]
                        for j in range(2):
                            nc.tensor.matmul(
                                xacc[:, j * 512:(j + 1) * 512],
                                lhs_v,
                                attst[:, j * 512:(j + 1) * 512],
                                start=(kk == 0), stop=(kk == NST - 1))
                    # normalize rows of this q-block by 1/rowsum and place
                    # into xw (natural [e, q] layout for the out-projection)
                    rcp = rn_pool.tile([1, QB], F32, tag="rcp", name="rcp")
                    nc.vector.reciprocal(rcp, xacc[DK:DK + 1, :])
                    rbc = rn_pool.tile([DK, QB], F32, tag="rbc", name="rbc")
                    nc.gpsimd.partition_broadcast(rbc, rcp)
                    nc.vector.tensor_tensor(
                        xw[et][hp:hp + DK, q0:q0 + QB],
                        xacc[0:DK, :], rbc, mybir.AluOpType.mult)
                # out-projection for this q block
                for qc in range(QC):
                    qq = q0 + qc * P
                    osb = osb_pool.tile([P, D], F16, tag="osb", name="osb")
                    for j in range(2):
                        oacc = ppool_w.tile([P, 512], F32, tag="oacc",
                                            name="oacc")
                        for ec in range(2):
                            nc.tensor.matmul(
                                oacc,
                                xw[ec][:, qq:qq + P],
                                w0sb[:, ec, j * 512:(j + 1) * 512],
                                start=(ec == 0), stop=(ec == 1))
                        nc.vector.tensor_copy(
                            out=osb[:, j * 512:(j + 1) * 512], in_=oacc)
                    nc.sync.dma_start(out=out[qq:qq + P, :], in_=osb)
        proj_ctx.close()


def build_program():
    nc = bacc.Bacc("TRN2", target_bir_lowering=False, debug=False,
                   num_devices=NCORES)
    qT = nc.dram_tensor("qT", (D, S), F16, kind="ExternalInput").ap()
    kT = nc.dram_tensor("kT", (D, S), F16, kind="ExternalInput").ap()
    vT = nc.dram_tensor("vT", (D, S), F16, kind="ExternalInput").ap()
    wqT = nc.dram_tensor("wqT", (D, E), F16, kind="ExternalInput").ap()
    wkT = nc.dram_tensor("wkT", (D, E), F16, kind="ExternalInput").ap()
    wvT = nc.dram_tensor("wvT", (D, E), F16, kind="ExternalInput").ap()
    w0T = nc.dram_tensor("w0T", (E, D), F16, kind="ExternalInput").ap()
    out = nc.dram_tensor("out", (S, D), F16, kind="ExternalOutput").ap()
    with tile.TileContext(nc) as tc:
        kernel_body(tc, qT, kT, vT, wqT, wkT, wvT, w0T, out)
    nc.compile()
    return nc


_NC_CACHE = None


def _get_program():
    global _NC_CACHE
    if _NC_CACHE is None:
        _NC_CACHE = build_program()
    return _NC_CACHE


def make_in_maps(q, k, v, wq, wk, wv, w0):
    arrs = [np.asarray(a, dtype=np.float32)
            for a in (q, k, v, wq, wk, wv, w0)]
    q, k, v, wq, wk, wv, w0 = arrs
    f16 = np.float16
    # per-batch transposed activations (shared by the 4 cores of a batch)
    qTb = [np.ascontiguousarray(q[b].T).astype(f16) for b in range(B)]
    kTb = [np.ascontiguousarray(k[b].T).astype(f16) for b in range(B)]
    vTb = [np.ascontiguousarray(v[b].T).astype(f16) for b in range(B)]
    in_maps = []
    for c in range(NCORES):
        b, g = c // GROUPS, c % GROUPS
        e0 = g * E
        in_maps.append({
            "qT": qTb[b],
            "kT": kTb[b],
            "vT": vTb[b],
            "wqT": np.ascontiguousarray(wq[e0:e0 + E, :].T).astype(f16),
            "wkT": np.ascontiguousarray(wk[e0:e0 + E, :].T).astype(f16),
            "wvT": np.ascontiguousarray(wv[e0:e0 + E, :].T).astype(f16),
            "w0T": np.ascontiguousarray(w0[:, e0:e0 + E].T).astype(f16),
        })
    return in_maps


def gather_out(results):
    out = np.zeros((B, S, D), dtype=np.float32)
    for c in range(NCORES):
        b = c // GROUPS
        out[b] += results[c]["out"].astype(np.float32)
    return out


def _install_ntff_hook_shim():
    """This image's antenv lacks axon_hooks; recreate it so trace=True works.

    Mirrors trn_agent_boot.trn_boot._ntff_profile_via_ctypes against
    /opt/axon/libaxon_pjrt.so.
    """
    import sys, types, ctypes, contextlib
    if "antenv.axon_hooks" in sys.modules:
        return
    mod = types.ModuleType("antenv.axon_hooks")
    mod._hook = None

    def set_axon_ntff_profile_hook(h):
        mod._hook = h

    def get_axon_ntff_profile_hook():
        return mod._hook

    mod.set_axon_ntff_profile_hook = set_axon_ntff_profile_hook
    mod.get_axon_ntff_profile_hook = get_axon_ntff_profile_hook
    sys.modules["antenv.axon_hooks"] = mod
    try:
        import antenv
        antenv.axon_hooks = mod
    except ImportError:
        pass

    so_path = "/opt/axon/libaxon_pjrt.so"
    try:
        lib = ctypes.CDLL(so_path)
        if not hasattr(lib, "axon_start_nrt_profile"):
            return
        lib.axon_start_nrt_profile.argtypes = [
            ctypes.POINTER(ctypes.c_int64), ctypes.c_size_t]
        lib.axon_start_nrt_profile.restype = ctypes.c_int64
        lib.axon_stop_nrt_profile.argtypes = [ctypes.c_char_p]
        lib.axon_stop_nrt_profile.restype = ctypes.c_int64
    except OSError:
        return

    @contextlib.contextmanager
    def _hook(output_dir, device_ids):
        import jax
        jax.devices()
        if device_ids:
            ids = (ctypes.c_int64 * len(device_ids))(*device_ids)
            rc = lib.axon_start_nrt_profile(ids, len(device_ids))
        else:
            rc = lib.axon_start_nrt_profile(None, 0)
        if rc != 0:
            raise RuntimeError(f"axon_start_nrt_profile rc={rc}")
        try:
            yield
        finally:
            n = lib.axon_stop_nrt_profile(str(output_dir).encode())
            print(f"profile: {n} file(s) written to {output_dir}")

    mod._hook = _hook


def kernel(q, k, v, wq, wk, wv, w0, _trace=False, _tmpdir=None):
    if _trace:
        _install_ntff_hook_shim()
    nc = _get_program()
    in_maps = make_in_maps(q, k, v, wq, wk, wv, w0)
    res = bass_utils.run_bass_kernel_spmd(
        nc, in_maps, core_ids=list(range(NCORES)),
        trace=_trace, tmpdir=_tmpdir)
    out = gather_out(res.results)
    if _trace:
        return out, res
    return out


# revision 12
# speedup vs baseline: 1.0840x; 1.0059x over previous
"""Multi-head attention (B=2, S=4096, D=1024, H=16) on 8 NeuronCores.

Sharding: core c = (batch b = c // 4, head-group g = c % 4).  Each head-group
owns 4 heads = 256 projection features.

v3: fp16 operands everywhere (single-pass PE matmuls, 10-bit mantissa keeps
rel err ~1e-3); all transposes/casts done on the HOST (inputs ship as
qT/kT/vT [D, S] fp16, wqT/wkT/wvT [D, E], w0T [E, D]); batched 3D-AP DMA
loads; softmax normalization fused into the PSUM->SBUF eviction of the PV
accumulator (reciprocal of the ones-column row, partition-broadcast,
tensor_tensor multiply) so there is no transpose-based normalize phase; the
output projection for a q-block is emitted right after its 4 heads finish,
overlapping the next block's attention.  Host sums the 4 fp16 partials per
batch in fp32.
"""

import numpy as np
from contextlib import ExitStack

import concourse.bass as bass
import concourse.bacc as bacc
import concourse.tile as tile
from concourse import mybir, bass_utils

B, S, D, H = 2, 4096, 1024, 16
DK = D // H          # 64
NCORES = 8
GROUPS = 4           # head-groups (tensor parallel)
HG = H // GROUPS     # 4 heads per group
E = HG * DK          # 256 features per group

F32 = mybir.dt.float32
F16 = mybir.dt.float16

P = 128              # partitions
SC = S // P          # 32 s-chunks of 128
SG = 8               # s-groups in projection
SGW = S // SG        # 512
DC = D // P          # 8 d-chunks
QB = 1024            # q-block in attention
NQB = S // QB        # 4
QC = QB // P         # 8 q-chunks per block
NST = SC             # 32 k-stripes of 128
VW = DK + 1          # vp columns per head incl. ones column (65)
VPAD = 66            # padded per-head stride in vps tile


def kernel_body(tc, qT, kT, vT, wqT, wkT, wvT, w0T, out):
    nc = tc.nc
    ctx = ExitStack()
    with ctx:
        # persistent weights (pre-transposed on host, loaded in 4 DMAs)
        w_pool = ctx.enter_context(tc.tile_pool(name="wsb", bufs=1))
        w0sb = w_pool.tile([P, 2, D], F16, tag="w0sb", name="w0sb")
        wqsb = w_pool.tile([P, DC, E], F16, tag="wqsb", name="wqsb")
        wksb = w_pool.tile([P, DC, E], F16, tag="wksb", name="wksb")
        wvsb = w_pool.tile([P, DC, E], F16, tag="wvsb", name="wvsb")
        nc.sync.dma_start(out=w0sb,
                          in_=w0T.rearrange("(c p) d -> p c d", p=P))
        for wsrc, wdst in ((wqT, wqsb), (wkT, wksb), (wvT, wvsb)):
            nc.sync.dma_start(out=wdst,
                              in_=wsrc.rearrange("(c p) e -> p c e", p=P))

        # persistent through phase A
        proj_ctx = ExitStack()
        proj_pool = proj_ctx.enter_context(tc.tile_pool(name="proj", bufs=1))
        qpT = [proj_pool.tile([P, S], F16, tag=f"qpT{i}", name=f"qpT{i}")
               for i in range(2)]
        kpT = [proj_pool.tile([P, S], F16, tag=f"kpT{i}", name=f"kpT{i}")
               for i in range(2)]
        vps = proj_pool.tile([P, SC, HG * VPAD], F16, tag="vps", name="vps")

        qTr = qT.rearrange("(c p) s -> p c s", p=P)
        kTr = kT.rearrange("(c p) s -> p c s", p=P)
        vTr = vT.rearrange("(c p) s -> p c s", p=P)

        # ================= phase P: projections =================
        with tc.tile_pool(name="p_xin", bufs=3) as xin_pool, \
             tc.tile_pool(name="p_psum", bufs=3, space="PSUM") as ppool_a:
            # k/v first: attention only needs kpT/vps in full plus the first
            # q-block of qpT, so q-projection for later blocks hides under
            # the ACT-bound start of attention
            for sg in range(SG):
                s0 = sg * SGW
                kg = xin_pool.tile([P, DC, SGW], F16, tag="kg", name="kg")
                vg = xin_pool.tile([P, DC, SGW], F16, tag="vg", name="vg")
                # split loads across two DGE queues so k and v stream in
                # parallel instead of serializing on the sync queue
                nc.sync.dma_start(out=kg, in_=kTr[:, :, s0:s0 + SGW])
                nc.scalar.dma_start(out=vg, in_=vTr[:, :, s0:s0 + SGW])
                for et in range(2):
                    acc = ppool_a.tile([P, SGW], F32, tag="acc", name="acc")
                    for dc in range(DC):
                        nc.tensor.matmul(
                            acc,
                            wksb[:, dc, et * P:(et + 1) * P],
                            kg[:, dc, :],
                            start=(dc == 0), stop=(dc == DC - 1))
                    nc.vector.tensor_copy(
                        out=kpT[et][:, s0:s0 + SGW], in_=acc)
                for sc4 in range(SGW // P):
                    scg = sg * (SGW // P) + sc4
                    accv = ppool_a.tile([P, E], F32, tag="accv", name="accv")
                    for dc in range(DC):
                        nc.tensor.matmul(
                            accv,
                            vg[:, dc, sc4 * P:(sc4 + 1) * P],
                            wvsb[:, dc, :],
                            start=(dc == 0), stop=(dc == DC - 1))
                    for h in range(HG):
                        nc.vector.tensor_copy(
                            out=vps[:, scg, h * VPAD:h * VPAD + DK],
                            in_=accv[:, h * DK:(h + 1) * DK])
            # ones column for the PV sums row
            ones_sc = xin_pool.tile([P, SC], F16, tag="ones_sc",
                                    name="ones_sc")
            nc.vector.memset(ones_sc, 1.0)
            for h in range(HG):
                nc.vector.tensor_copy(
                    out=vps[:, :, h * VPAD + DK:h * VPAD + DK + 1],
                    in_=ones_sc.rearrange("p (s o) -> p s o", o=1))
            for sg in range(SG):
                s0 = sg * SGW
                qg = xin_pool.tile([P, DC, SGW], F16, tag="qg", name="qg")
                nc.sync.dma_start(out=qg, in_=qTr[:, :, s0:s0 + SGW])
                for et in range(2):
                    acc = ppool_a.tile([P, SGW], F32, tag="acc", name="acc")
                    for dc in range(DC):
                        nc.tensor.matmul(
                            acc,
                            wqsb[:, dc, et * P:(et + 1) * P],
                            qg[:, dc, :],
                            start=(dc == 0), stop=(dc == DC - 1))
                    nc.vector.tensor_copy(
                        out=qpT[et][:, s0:s0 + SGW], in_=acc)

        # ============ phase A: attention + interleaved out-proj ============
        xw_pool = ctx.enter_context(
            tc.tile_pool(name="xw", bufs=1, side="right"))
        xw = [xw_pool.tile([P, S], F16, tag=f"xw{i}", name=f"xw{i}")
              for i in range(2)]
        with tc.tile_pool(name="a_att", bufs=3) as att_pool, \
             tc.tile_pool(name="a_rn", bufs=2) as rn_pool, \
             tc.tile_pool(name="a_osb", bufs=2) as osb_pool, \
             tc.tile_pool(name="a_st", bufs=2, space="PSUM") as ppool_st, \
             tc.tile_pool(name="a_x", bufs=1, space="PSUM") as ppool_x, \
             tc.tile_pool(name="a_w", bufs=2, space="PSUM") as ppool_w:
            def emit_wproj(qb, qc):
                # out-projection for q-chunk qc of block qb (xw already final)
                qq = qb * QB + qc * P
                osb = osb_pool.tile([P, D], F16, tag="osb", name="osb")
                for j in range(2):
                    oacc = ppool_w.tile([P, 512], F32, tag="oacc",
                                        name="oacc")
                    for ec in range(2):
                        nc.tensor.matmul(
                            oacc,
                            xw[ec][:, qq:qq + P],
                            w0sb[:, ec, j * 512:(j + 1) * 512],
                            start=(ec == 0), stop=(ec == 1))
                    nc.vector.tensor_copy(
                        out=osb[:, j * 512:(j + 1) * 512], in_=oacc)
                nc.sync.dma_start(out=out[qq:qq + P, :], in_=osb)

            for qb in range(NQB):
                q0 = qb * QB
                for h in range(HG):
                    et, hp = h // 2, (h % 2) * DK
                    xacc = ppool_x.tile([VW, QB], F32, tag="xacc",
                                        name="xacc")
                    for kk in range(NST):
                        attst = att_pool.tile([P, QB], F16, tag="att",
                                              name="att")
                        lhs_k = kpT[et][hp:hp + DK, kk * P:(kk + 1) * P]
                        st = ppool_st.tile([P, QB], F32, tag="st", name="st")
                        for j in range(2):
                            nc.tensor.matmul(
                                st[:, j * 512:(j + 1) * 512],
                                lhs_k,
                                qpT[et][hp:hp + DK,
                                        q0 + j * 512:q0 + (j + 1) * 512],
                                start=True, stop=True)
                        nc.scalar.activation(
                            attst, st, mybir.ActivationFunctionType.Exp,
                            scale=0.125)
                        lhs_v = vps[:, kk, h * VPAD:h * VPAD + VW]
                        for j in range(2):
                            nc.tensor.matmul(
                                xacc[:, j * 512:(j + 1) * 512],
                                lhs_v,
                                attst[:, j * 512:(j + 1) * 512],
                                start=(kk == 0), stop=(kk == NST - 1))
                        # spread the previous block's out-projection across
                        # k-stripes of the first head instead of one burst
                        # that would leave the ACT engine idle
                        if h == 0 and qb > 0 and kk % 4 == 3:
                            emit_wproj(qb - 1, kk // 4)
                    # normalize rows of this q-block by 1/rowsum and place
                    # into xw (natural [e, q] layout for the out-projection)
                    rcp = rn_pool.tile([1, QB], F32, tag="rcp", name="rcp")
                    nc.vector.reciprocal(rcp, xacc[DK:DK + 1, :])
                    rbc = rn_pool.tile([DK, QB], F32, tag="rbc", name="rbc")
                    nc.gpsimd.partition_broadcast(rbc, rcp)
                    nc.vector.tensor_tensor(
                        xw[et][hp:hp + DK, q0:q0 + QB],
                        xacc[0:DK, :], rbc, mybir.AluOpType.mult)
            for qc in range(QC):
                emit_wproj(NQB - 1, qc)
        proj_ctx.close()


def build_program():
    nc = bacc.Bacc("TRN2", target_bir_lowering=False, debug=False,
                   num_devices=NCORES)
    qT = nc.dram_tensor("qT", (D, S), F16, kind="ExternalInput").ap()
    kT = nc.dram_tensor("kT", (D, S), F16, kind="ExternalInput").ap()
    vT = nc.dram_tensor("vT", (D, S), F16, kind="ExternalInput").ap()
    wqT = nc.dram_tensor("wqT", (D, E), F16, kind="ExternalInput").ap()
    wkT = nc.dram_tensor("wkT", (D, E), F16, kind="ExternalInput").ap()
    wvT = nc.dram_tensor("wvT", (D, E), F16, kind="ExternalInput").ap()
    w0T = nc.dram_tensor("w0T", (E, D), F16, kind="ExternalInput").ap()
    out = nc.dram_tensor("out", (S, D), F16, kind="ExternalOutput").ap()
    with tile.TileContext(nc) as tc:
        kernel_body(tc, qT, kT, vT, wqT, wkT, wvT, w0T, out)
    nc.compile()
    return nc


_NC_CACHE = None


def _get_program():
    global _NC_CACHE
    if _NC_CACHE is None:
        _NC_CACHE = build_program()
    return _NC_CACHE


def make_in_maps(q, k, v, wq, wk, wv, w0):
    arrs = [np.asarray(a, dtype=np.float32)
            for a in (q, k, v, wq, wk, wv, w0)]
    q, k, v, wq, wk, wv, w0 = arrs
    f16 = np.float16
    # per-batch transposed activations (shared by the 4 cores of a batch)
    qTb = [np.ascontiguousarray(q[b].T).astype(f16) for b in range(B)]
    kTb = [np.ascontiguousarray(k[b].T).astype(f16) for b in range(B)]
    vTb = [np.ascontiguousarray(v[b].T).astype(f16) for b in range(B)]
    in_maps = []
    for c in range(NCORES):
        b, g = c // GROUPS, c % GROUPS
        e0 = g * E
        in_maps.append({
            "qT": qTb[b],
            "kT": kTb[b],
            "vT": vTb[b],
            "wqT": np.ascontiguousarray(wq[e0:e0 + E, :].T).astype(f16),
            "wkT": np.ascontiguousarray(wk[e0:e0 + E, :].T).astype(f16),
            "wvT": np.ascontiguousarray(wv[e0:e0 + E, :].T).astype(f16),
            "w0T": np.ascontiguousarray(w0[:, e0:e0 + E].T).astype(f16),
        })
    return in_maps


def gather_out(results):
    out = np.zeros((B, S, D), dtype=np.float32)
    for c in range(NCORES):
        b = c // GROUPS
        out[b] += results[c]["out"].astype(np.float32)
    return out


def _install_ntff_hook_shim():
    """This image's antenv lacks axon_hooks; recreate it so trace=True works.

    Mirrors trn_agent_boot.trn_boot._ntff_profile_via_ctypes against
    /opt/axon/libaxon_pjrt.so.
    """
    import sys, types, ctypes, contextlib
    if "antenv.axon_hooks" in sys.modules:
        return
    mod = types.ModuleType("antenv.axon_hooks")
    mod._hook = None

    def set_axon_ntff_profile_hook(h):
        mod._hook = h

    def get_axon_ntff_profile_hook():
        return mod._hook

    mod.set_axon_ntff_profile_hook = set_axon_ntff_profile_hook
    mod.get_axon_ntff_profile_hook = get_axon_ntff_profile_hook
    sys.modules["antenv.axon_hooks"] = mod
    try:
        import antenv
        antenv.axon_hooks = mod
    except ImportError:
        pass

    so_path = "/opt/axon/libaxon_pjrt.so"
    try:
        lib = ctypes.CDLL(so_path)
        if not hasattr(lib, "axon_start_nrt_profile"):
            return
        lib.axon_start_nrt_profile.argtypes = [
            ctypes.POINTER(ctypes.c_int64), ctypes.c_size_t]
        lib.axon_start_nrt_profile.restype = ctypes.c_int64
        lib.axon_stop_nrt_profile.argtypes = [ctypes.c_char_p]
        lib.axon_stop_nrt_profile.restype = ctypes.c_int64
    except OSError:
        return

    @contextlib.contextmanager
    def _hook(output_dir, device_ids):
        import jax
        jax.devices()
        if device_ids:
            ids = (ctypes.c_int64 * len(device_ids))(*device_ids)
            rc = lib.axon_start_nrt_profile(ids, len(device_ids))
        else:
            rc = lib.axon_start_nrt_profile(None, 0)
        if rc != 0:
            raise RuntimeError(f"axon_start_nrt_profile rc={rc}")
        try:
            yield
        finally:
            n = lib.axon_stop_nrt_profile(str(output_dir).encode())
            print(f"profile: {n} file(s) written to {output_dir}")

    mod._hook = _hook


def kernel(q, k, v, wq, wk, wv, w0, _trace=False, _tmpdir=None):
    if _trace:
        _install_ntff_hook_shim()
    nc = _get_program()
    in_maps = make_in_maps(q, k, v, wq, wk, wv, w0)
    res = bass_utils.run_bass_kernel_spmd(
        nc, in_maps, core_ids=list(range(NCORES)),
        trace=_trace, tmpdir=_tmpdir)
    out = gather_out(res.results)
    if _trace:
        return out, res
    return out


# revision 14
# speedup vs baseline: 1.0852x; 1.0010x over previous
"""Multi-head attention (B=2, S=4096, D=1024, H=16) on 8 NeuronCores.

Sharding: core c = (batch b = c // 4, head-group g = c % 4).  Each head-group
owns 4 heads = 256 projection features.

v3: fp16 operands everywhere (single-pass PE matmuls, 10-bit mantissa keeps
rel err ~1e-3); all transposes/casts done on the HOST (inputs ship as
qT/kT/vT [D, S] fp16, wqT/wkT/wvT [D, E], w0T [E, D]); batched 3D-AP DMA
loads; softmax normalization fused into the PSUM->SBUF eviction of the PV
accumulator (reciprocal of the ones-column row, partition-broadcast,
tensor_tensor multiply) so there is no transpose-based normalize phase; the
output projection for a q-block is emitted right after its 4 heads finish,
overlapping the next block's attention.  Host sums the 4 fp16 partials per
batch in fp32.
"""

import numpy as np
from contextlib import ExitStack

import concourse.bass as bass
import concourse.bacc as bacc
import concourse.tile as tile
from concourse import mybir, bass_utils

B, S, D, H = 2, 4096, 1024, 16
DK = D // H          # 64
NCORES = 8
GROUPS = 4           # head-groups (tensor parallel)
HG = H // GROUPS     # 4 heads per group
E = HG * DK          # 256 features per group

F32 = mybir.dt.float32
F16 = mybir.dt.float16

P = 128              # partitions
SC = S // P          # 32 s-chunks of 128
SG = 8               # s-groups in projection
SGW = S // SG        # 512
DC = D // P          # 8 d-chunks
QB = 1024            # q-block in attention
NQB = S // QB        # 4
QC = QB // P         # 8 q-chunks per block
NST = SC             # 32 k-stripes of 128
VW = DK + 1          # vp columns per head incl. ones column (65)
VPAD = 66            # padded per-head stride in vps tile


def kernel_body(tc, qT, kT, vT, wqT, wkT, wvT, w0T, out):
    nc = tc.nc
    ctx = ExitStack()
    with ctx:
        # persistent weights (pre-transposed on host, loaded in 4 DMAs)
        w_pool = ctx.enter_context(tc.tile_pool(name="wsb", bufs=1))
        w0sb = w_pool.tile([P, 2, D], F16, tag="w0sb", name="w0sb")
        wqsb = w_pool.tile([P, DC, E], F16, tag="wqsb", name="wqsb")
        wksb = w_pool.tile([P, DC, E], F16, tag="wksb", name="wksb")
        wvsb = w_pool.tile([P, DC, E], F16, tag="wvsb", name="wvsb")
        nc.sync.dma_start(out=w0sb,
                          in_=w0T.rearrange("(c p) d -> p c d", p=P))
        for wsrc, wdst in ((wqT, wqsb), (wkT, wksb), (wvT, wvsb)):
            nc.sync.dma_start(out=wdst,
                              in_=wsrc.rearrange("(c p) e -> p c e", p=P))

        # persistent through phase A
        proj_ctx = ExitStack()
        proj_pool = proj_ctx.enter_context(tc.tile_pool(name="proj", bufs=1))
        qpT = [proj_pool.tile([P, S], F16, tag=f"qpT{i}", name=f"qpT{i}")
               for i in range(2)]
        kpT = [proj_pool.tile([P, S], F16, tag=f"kpT{i}", name=f"kpT{i}")
               for i in range(2)]
        vps = proj_pool.tile([P, SC, HG * VPAD], F16, tag="vps", name="vps")

        qTr = qT.rearrange("(c p) s -> p c s", p=P)
        kTr = kT.rearrange("(c p) s -> p c s", p=P)
        vTr = vT.rearrange("(c p) s -> p c s", p=P)

        # ================= phase P: projections =================
        with tc.tile_pool(name="p_xin", bufs=2) as xin_pool, \
             tc.tile_pool(name="p_psum", bufs=3, space="PSUM") as ppool_a:
            # k/v first: attention only needs kpT/vps in full plus the first
            # q-block of qpT, so q-projection for later blocks hides under
            # the ACT-bound start of attention.  Loads are 1024 columns wide
            # and k/v go to different DGE queues so the serial DMA prefix is
            # short.
            for sg in range(SG // 2):
                s0 = sg * 2 * SGW
                kg = xin_pool.tile([P, DC, 2 * SGW], F16, tag="kg", name="kg")
                vg = xin_pool.tile([P, DC, 2 * SGW], F16, tag="vg", name="vg")
                nc.sync.dma_start(out=kg, in_=kTr[:, :, s0:s0 + 2 * SGW])
                nc.scalar.dma_start(out=vg, in_=vTr[:, :, s0:s0 + 2 * SGW])
                for half in range(2):
                    h0 = half * SGW
                    for et in range(2):
                        acc = ppool_a.tile([P, SGW], F32, tag="acc",
                                           name="acc")
                        for dc in range(DC):
                            nc.tensor.matmul(
                                acc,
                                wksb[:, dc, et * P:(et + 1) * P],
                                kg[:, dc, h0:h0 + SGW],
                                start=(dc == 0), stop=(dc == DC - 1))
                        nc.vector.tensor_copy(
                            out=kpT[et][:, s0 + h0:s0 + h0 + SGW], in_=acc)
                    for sc4 in range(SGW // P):
                        scg = (s0 + h0) // P + sc4
                        accv = ppool_a.tile([P, E], F32, tag="accv",
                                            name="accv")
                        for dc in range(DC):
                            nc.tensor.matmul(
                                accv,
                                vg[:, dc, h0 + sc4 * P:h0 + (sc4 + 1) * P],
                                wvsb[:, dc, :],
                                start=(dc == 0), stop=(dc == DC - 1))
                        for h in range(HG):
                            nc.vector.tensor_copy(
                                out=vps[:, scg, h * VPAD:h * VPAD + DK],
                                in_=accv[:, h * DK:(h + 1) * DK])
            # ones column for the PV sums row
            ones_sc = xin_pool.tile([P, SC], F16, tag="ones_sc",
                                    name="ones_sc")
            nc.vector.memset(ones_sc, 1.0)
            for h in range(HG):
                nc.vector.tensor_copy(
                    out=vps[:, :, h * VPAD + DK:h * VPAD + DK + 1],
                    in_=ones_sc.rearrange("p (s o) -> p s o", o=1))
            for sg in range(SG // 2):
                s0 = sg * 2 * SGW
                qg = xin_pool.tile([P, DC, 2 * SGW], F16, tag="qg", name="qg")
                nc.sync.dma_start(out=qg, in_=qTr[:, :, s0:s0 + 2 * SGW])
                for half in range(2):
                    h0 = half * SGW
                    for et in range(2):
                        acc = ppool_a.tile([P, SGW], F32, tag="acc",
                                           name="acc")
                        for dc in range(DC):
                            nc.tensor.matmul(
                                acc,
                                wqsb[:, dc, et * P:(et + 1) * P],
                                qg[:, dc, h0:h0 + SGW],
                                start=(dc == 0), stop=(dc == DC - 1))
                        nc.vector.tensor_copy(
                            out=qpT[et][:, s0 + h0:s0 + h0 + SGW], in_=acc)

        # ============ phase A: attention + interleaved out-proj ============
        xw_pool = ctx.enter_context(
            tc.tile_pool(name="xw", bufs=1, side="right"))
        xw = [xw_pool.tile([P, S], F16, tag=f"xw{i}", name=f"xw{i}")
              for i in range(2)]
        with tc.tile_pool(name="a_att", bufs=3) as att_pool, \
             tc.tile_pool(name="a_rn", bufs=2) as rn_pool, \
             tc.tile_pool(name="a_osb", bufs=2) as osb_pool, \
             tc.tile_pool(name="a_st", bufs=2, space="PSUM") as ppool_st, \
             tc.tile_pool(name="a_x", bufs=1, space="PSUM") as ppool_x, \
             tc.tile_pool(name="a_w", bufs=2, space="PSUM") as ppool_w:
            def emit_wproj(qb, qc):
                # out-projection for q-chunk qc of block qb (xw already final)
                qq = qb * QB + qc * P
                osb = osb_pool.tile([P, D], F16, tag="osb", name="osb")
                for j in range(2):
                    oacc = ppool_w.tile([P, 512], F32, tag="oacc",
                                        name="oacc")
                    for ec in range(2):
                        nc.tensor.matmul(
                            oacc,
                            xw[ec][:, qq:qq + P],
                            w0sb[:, ec, j * 512:(j + 1) * 512],
                            start=(ec == 0), stop=(ec == 1))
                    nc.vector.tensor_copy(
                        out=osb[:, j * 512:(j + 1) * 512], in_=oacc)
                nc.sync.dma_start(out=out[qq:qq + P, :], in_=osb)

            for qb in range(NQB):
                q0 = qb * QB
                for h in range(HG):
                    et, hp = h // 2, (h % 2) * DK
                    xacc = ppool_x.tile([VW, QB], F32, tag="xacc",
                                        name="xacc")
                    for kk in range(NST):
                        attst = att_pool.tile([P, QB], F16, tag="att",
                                              name="att")
                        lhs_k = kpT[et][hp:hp + DK, kk * P:(kk + 1) * P]
                        st = ppool_st.tile([P, QB], F32, tag="st", name="st")
                        for j in range(2):
                            nc.tensor.matmul(
                                st[:, j * 512:(j + 1) * 512],
                                lhs_k,
                                qpT[et][hp:hp + DK,
                                        q0 + j * 512:q0 + (j + 1) * 512],
                                start=True, stop=True)
                        nc.scalar.activation(
                            attst, st, mybir.ActivationFunctionType.Exp,
                            scale=0.125)
                        lhs_v = vps[:, kk, h * VPAD:h * VPAD + VW]
                        for j in range(2):
                            nc.tensor.matmul(
                                xacc[:, j * 512:(j + 1) * 512],
                                lhs_v,
                                attst[:, j * 512:(j + 1) * 512],
                                start=(kk == 0), stop=(kk == NST - 1))
                        # spread the previous block's out-projection across
                        # k-stripes of the first head instead of one burst
                        # that would leave the ACT engine idle
                        if h == 0 and qb > 0 and kk % 4 == 3:
                            emit_wproj(qb - 1, kk // 4)
                    # normalize rows of this q-block by 1/rowsum and place
                    # into xw (natural [e, q] layout for the out-projection)
                    rcp = rn_pool.tile([1, QB], F32, tag="rcp", name="rcp")
                    nc.vector.reciprocal(rcp, xacc[DK:DK + 1, :])
                    rbc = rn_pool.tile([DK, QB], F32, tag="rbc", name="rbc")
                    nc.gpsimd.partition_broadcast(rbc, rcp)
                    nc.vector.tensor_tensor(
                        xw[et][hp:hp + DK, q0:q0 + QB],
                        xacc[0:DK, :], rbc, mybir.AluOpType.mult)
            for qc in range(QC):
                emit_wproj(NQB - 1, qc)
        proj_ctx.close()


def build_program():
    nc = bacc.Bacc("TRN2", target_bir_lowering=False, debug=False,
                   num_devices=NCORES)
    qT = nc.dram_tensor("qT", (D, S), F16, kind="ExternalInput").ap()
    kT = nc.dram_tensor("kT", (D, S), F16, kind="ExternalInput").ap()
    vT = nc.dram_tensor("vT", (D, S), F16, kind="ExternalInput").ap()
    wqT = nc.dram_tensor("wqT", (D, E), F16, kind="ExternalInput").ap()
    wkT = nc.dram_tensor("wkT", (D, E), F16, kind="ExternalInput").ap()
    wvT = nc.dram_tensor("wvT", (D, E), F16, kind="ExternalInput").ap()
    w0T = nc.dram_tensor("w0T", (E, D), F16, kind="ExternalInput").ap()
    out = nc.dram_tensor("out", (S, D), F16, kind="ExternalOutput").ap()
    with tile.TileContext(nc) as tc:
        kernel_body(tc, qT, kT, vT, wqT, wkT, wvT, w0T, out)
    nc.compile()
    return nc


_NC_CACHE = None


def _get_program():
    global _NC_CACHE
    if _NC_CACHE is None:
        _NC_CACHE = build_program()
    return _NC_CACHE


def make_in_maps(q, k, v, wq, wk, wv, w0):
    arrs = [np.asarray(a, dtype=np.float32)
            for a in (q, k, v, wq, wk, wv, w0)]
    q, k, v, wq, wk, wv, w0 = arrs
    f16 = np.float16
    # per-batch transposed activations (shared by the 4 cores of a batch)
    qTb = [np.ascontiguousarray(q[b].T).astype(f16) for b in range(B)]
    kTb = [np.ascontiguousarray(k[b].T).astype(f16) for b in range(B)]
    vTb = [np.ascontiguousarray(v[b].T).astype(f16) for b in range(B)]
    in_maps = []
    for c in range(NCORES):
        b, g = c // GROUPS, c % GROUPS
        e0 = g * E
        in_maps.append({
            "qT": qTb[b],
            "kT": kTb[b],
            "vT": vTb[b],
            "wqT": np.ascontiguousarray(wq[e0:e0 + E, :].T).astype(f16),
            "wkT": np.ascontiguousarray(wk[e0:e0 + E, :].T).astype(f16),
            "wvT": np.ascontiguousarray(wv[e0:e0 + E, :].T).astype(f16),
            "w0T": np.ascontiguousarray(w0[:, e0:e0 + E].T).astype(f16),
        })
    return in_maps


def gather_out(results):
    out = np.zeros((B, S, D), dtype=np.float32)
    for c in range(NCORES):
        b = c // GROUPS
        out[b] += results[c]["out"].astype(np.float32)
    return out


def _install_ntff_hook_shim():
    """This image's antenv lacks axon_hooks; recreate it so trace=True works.

    Mirrors trn_agent_boot.trn_boot._ntff_profile_via_ctypes against
    /opt/axon/libaxon_pjrt.so.
    """
    import sys, types, ctypes, contextlib
    if "antenv.axon_hooks" in sys.modules:
        return
    mod = types.ModuleType("antenv.axon_hooks")
    mod._hook = None

    def set_axon_ntff_profile_hook(h):
        mod._hook = h

    def get_axon_ntff_profile_hook():
        return mod._hook

    mod.set_axon_ntff_profile_hook = set_axon_ntff_profile_hook
    mod.get_axon_ntff_profile_hook = get_axon_ntff_profile_hook
    sys.modules["antenv.axon_hooks"] = mod
    try:
        import antenv
        antenv.axon_hooks = mod
    except ImportError:
        pass

    so_path = "/opt/axon/libaxon_pjrt.so"
    try:
        lib = ctypes.CDLL(so_path)
        if not hasattr(lib, "axon_start_nrt_profile"):
            return
        lib.axon_start_nrt_profile.argtypes = [
            ctypes.POINTER(ctypes.c_int64), ctypes.c_size_t]
        lib.axon_start_nrt_profile.restype = ctypes.c_int64
        lib.axon_stop_nrt_profile.argtypes = [ctypes.c_char_p]
        lib.axon_stop_nrt_profile.restype = ctypes.c_int64
    except OSError:
        return

    @contextlib.contextmanager
    def _hook(output_dir, device_ids):
        import jax
        jax.devices()
        if device_ids:
            ids = (ctypes.c_int64 * len(device_ids))(*device_ids)
            rc = lib.axon_start_nrt_profile(ids, len(device_ids))
        else:
            rc = lib.axon_start_nrt_profile(None, 0)
        if rc != 0:
            raise RuntimeError(f"axon_start_nrt_profile rc={rc}")
        try:
            yield
        finally:
            n = lib.axon_stop_nrt_profile(str(output_dir).encode())
            print(f"profile: {n} file(s) written to {output_dir}")

    mod._hook = _hook


def kernel(q, k, v, wq, wk, wv, w0, _trace=False, _tmpdir=None):
    if _trace:
        _install_ntff_hook_shim()
    nc = _get_program()
    in_maps = make_in_maps(q, k, v, wq, wk, wv, w0)
    res = bass_utils.run_bass_kernel_spmd(
        nc, in_maps, core_ids=list(range(NCORES)),
        trace=_trace, tmpdir=_tmpdir)
    out = gather_out(res.results)
    if _trace:
        return out, res
    return out
